# revision 27
# baseline (speedup 1.0000x reference)
"""Trainium2 Bass kernel for nn_Attention_1992864825947.

Sharding: pure data-parallel over batch (B=8 -> one batch per NeuronCore,
zero collectives).  Each core runs the complete attention block for its
batch; the host shards inputs / gathers outputs.

Layout + precision strategy:
- Host passes pre-transposed, bf16-pre-cast copies of the matmul inputs
  (memory^T, decoder^T, W*^T) - TRN2 fp32 matmul costs 4 cycles/row (two
  half-speed passes) while bf16 costs 1, and PE transposes of raw inputs
  would burn ~70k cycles/core.  Softmax, LayerNorm and both outputs stay
  fp32 in compute (attention DRAM tensor is bf16, widened exactly on host).
- All matmuls contract over the partition axis; softmax/LayerNorm reduce
  over the free axis:
    K^T = WK @ M^T, Q^T = (WQ @ D^T)/sqrt(d), V = M @ WV^T     (PE bf16)
    scores[sq,sk] = Q^T.T @ K^T  per head, PSUM fp32          (PE bf16)
    masked = mask_u8*(-2^32) + scores   (one DVE scalar_tensor_tensor)
    exp + row-sums in one ACT pass (accum_out); no max-subtraction
      needed (scores are bounded ~N(0,1) after the 1/sqrt(d) fold)
    attn_bf16 = exp * (query_mask/sum)  (per-partition scalar, DVE)
    attn^T via PE transposes (bf16, 4 tiles batched per PSUM tile)
    attn @ V with V stationary, accumulated over sk tiles      (PE bf16)
    result = concat(D, attn_out) @ Wf^T + bf + D, then LayerNorm with
      fused residual+rowsum (scalar_tensor_tensor accum_out) and
      rstd/mean folded into one ACT Identity pass.

Measured on trn2 (8 cores, NTFF profile): ~265 us, rel err ~4.6e-3
(bf16-dominated; gate 2e-2).
"""

import numpy as np
from contextlib import ExitStack

import concourse.bass as bass
import concourse.tile as tile
from concourse import bacc, mybir
from concourse.bass_utils import run_bass_kernel_spmd
from concourse.masks import make_identity

F32 = mybir.dt.float32
BF16 = mybir.dt.bfloat16
U8 = mybir.dt.uint8
AF = mybir.ActivationFunctionType
OP = mybir.AluOpType

S = 1024          # sequence length
H = 768           # hidden
NH = 4            # heads
DH = H // NH      # 192 head dim
ST = S // 128     # 8 seq tiles
HT = H // 128     # 6 hidden tiles
ZT = 2 * HT       # 12 tiles of concat dim
BIG = float(2 ** 32)
SCALE = 1.0 / float(np.sqrt(DH))
LN_EPS = 1e-5
N_CORES = 8


def _head_segs(h):
    """Partition-tile segments covering rows [h*DH, (h+1)*DH) of a
    [H, S]-shaped tensor stored as HT tiles of 128 partitions."""
    segs = []
    r = h * DH
    end = (h + 1) * DH
    while r < end:
        t = r // 128
        lo = r % 128
        hi = min(128, lo + (end - r))
        segs.append((t, lo, hi))
        r += hi - lo
    return segs


def _copy_zl_rows(nc, zl, lstart, src, nrows, col_off, width, engine):
    """Copy src[0:nrows, 0:width] (PSUM f32) into logical rows
    [lstart, lstart+nrows) of the Zt-lower tiles (bf16)."""
    r = 0
    while r < nrows:
        t = (lstart + r) // 128
        lo = (lstart + r) % 128
        n = min(128 - lo, nrows - r)
        dst = zl[t][lo:lo + n, col_off:col_off + width]
        if engine == "v":
            nc.vector.tensor_copy(dst, src[r:r + n, 0:width])
        else:
            nc.scalar.copy(dst, src[r:r + n, 0:width])
        r += n


def build_kernel():
    nc = bacc.Bacc("TRN2", target_bir_lowering=False, debug=False,
                   num_devices=N_CORES)
    mem_t = nc.dram_tensor("memory_T", [H, S], BF16, kind="ExternalInput").ap()
    dec_t = nc.dram_tensor("decoder_T", [H, S], BF16, kind="ExternalInput").ap()
    dec = nc.dram_tensor("decoder_input", [S, H], F32, kind="ExternalInput").ap()
    msk = nc.dram_tensor("mask", [S, S], U8, kind="ExternalInput").ap()
    qm = nc.dram_tensor("query_mask", [S], F32, kind="ExternalInput").ap()
    wk_t = nc.dram_tensor("WK_T", [H, H], BF16, kind="ExternalInput").ap()
    wv_t = nc.dram_tensor("WV_T", [H, H], BF16, kind="ExternalInput").ap()
    wq_t = nc.dram_tensor("WQ_T", [H, H], BF16, kind="ExternalInput").ap()
    wf_t = nc.dram_tensor("Wf_T", [2 * H, H], BF16, kind="ExternalInput").ap()
    bfv = nc.dram_tensor("bf", [H], BF16, kind="ExternalInput").ap()
    gav = nc.dram_tensor("gamma", [H], BF16, kind="ExternalInput").ap()
    bev = nc.dram_tensor("beta", [H], BF16, kind="ExternalInput").ap()
    out_res = nc.dram_tensor("out_res", [S, H], F32, kind="ExternalOutput").ap()
    out_att = nc.dram_tensor("out_att", [NH, S, S], BF16, kind="ExternalOutput").ap()

    with tile.TileContext(nc) as tc, ExitStack() as top:
        persist = top.enter_context(tc.tile_pool(name="persist", bufs=1))
        ps_big = top.enter_context(tc.tile_pool(name="ps_big", bufs=2, space="PSUM"))
        ps_av = top.enter_context(tc.tile_pool(name="ps_av", bufs=1, space="PSUM"))
        ps_tr = top.enter_context(tc.tile_pool(name="ps_tr", bufs=2, space="PSUM"))

        ident = persist.tile([128, 128], F32, tag="ident")
        make_identity(nc, ident[:])
        ident_bf = persist.tile([128, 128], BF16, tag="identbf")
        nc.vector.tensor_copy(ident_bf[:], ident[:])
        qm_sb = persist.tile([128, ST], F32, tag="qm")
        nc.sync.dma_start(qm_sb[:], qm.rearrange("(j p) -> p j", p=128))
        dtt = [persist.tile([128, S], BF16, tag=f"dtt{i}", name=f"dtt{i}")
               for i in range(HT)]
        zl = [persist.tile([128, S], BF16, tag=f"zl{i}", name=f"zl{i}")
              for i in range(HT)]
        for i in range(HT):
            for lo in (0, 64):
                nc.sync.dma_start(dtt[i][lo:lo + 64, :],
                                  dec_t[i * 128 + lo:i * 128 + lo + 64, :])

        # Phase C tensors (pools opened before kqv for LIFO release order;
        # loads issued at phase B start as prefetch)
        wfp = top.enter_context(tc.tile_pool(name="wfp", bufs=1))
        d2p = top.enter_context(tc.tile_pool(name="d2p", bufs=1))
        bcp = top.enter_context(tc.tile_pool(name="bcp", bufs=1))
        wft = [wfp.tile([128, H], BF16, tag=f"wft{i}", name=f"wft{i}")
               for i in range(ZT)]
        d2 = [d2p.tile([128, H], F32, tag=f"d2{i}", name=f"d2{i}")
              for i in range(ST)]
        vecs = {}
        for nm, vap in (("bf", bfv), ("ga", gav), ("be", bev)):
            vec = bcp.tile([1, H], BF16, tag=f"vec{nm}", name=f"vec{nm}")
            nc.sync.dma_start(vec[:], vap.rearrange("(p h) -> p h", p=1))
            vecs[nm] = vec
        ones = bcp.tile([1, 128], BF16, tag="ones")
        nc.gpsimd.memset(ones[:], 1.0)

        with ExitStack() as kqv_ctx:
            kqv = kqv_ctx.enter_context(tc.tile_pool(name="kqv", bufs=1))
            kt = [kqv.tile([128, S], BF16, tag=f"kt{i}", name=f"kt{i}")
                  for i in range(HT)]
            qt = [kqv.tile([128, S], BF16, tag=f"qt{i}", name=f"qt{i}")
                  for i in range(HT)]
            v = [kqv.tile([128, H], BF16, tag=f"v{i}", name=f"v{i}")
                 for i in range(ST)]

            # ---------------- Phase A: projections -----------------------
            with ExitStack() as pa:
                mtp = pa.enter_context(tc.tile_pool(name="mtp", bufs=1))
                mt = [mtp.tile([128, S], BF16, tag=f"mt{i}", name=f"mt{i}")
                      for i in range(HT)]
                wkt = [mtp.tile([128, H], BF16, tag=f"wkt{i}", name=f"wkt{i}")
                       for i in range(HT)]
                wqt = [mtp.tile([128, H], BF16, tag=f"wqt{i}", name=f"wqt{i}")
                       for i in range(HT)]
                wvt = [mtp.tile([128, H], BF16, tag=f"wvt{i}", name=f"wvt{i}")
                       for i in range(HT)]
                for i in range(HT):
                    for lo in (0, 64):
                        nc.sync.dma_start(wkt[i][lo:lo + 64, :],
                                          wk_t[i * 128 + lo:i * 128 + lo + 64, :])
                        nc.sync.dma_start(mt[i][lo:lo + 64, :],
                                          mem_t[i * 128 + lo:i * 128 + lo + 64, :])
                for i in range(HT):
                    for lo in (0, 64):
                        nc.sync.dma_start(wqt[i][lo:lo + 64, :],
                                          wq_t[i * 128 + lo:i * 128 + lo + 64, :])
                for i in range(HT):
                    nc.sync.dma_start(wvt[i][:], wv_t[i * 128:(i + 1) * 128, :])

                # K^T/Q^T [o, s] = W @ X^T: lhsT = Wt (h,o), rhs = Xt (h,s)
                def project(wt, src, dst, scale, ot):
                    o_ps = ps_big.tile([128, S], F32, tag="bigps", name="o_ps")
                    for ht in range(HT):
                        for c0 in (0, 512):
                            nc.tensor.matmul(
                                o_ps[:, c0:c0 + 512],
                                wt[ht][:, ot * 128:(ot + 1) * 128],
                                src[ht][:, c0:c0 + 512],
                                start=(ht == 0), stop=(ht == HT - 1))
                    if scale is None:
                        nc.scalar.copy(dst[ot][:], o_ps[:])
                    else:
                        nc.scalar.mul(dst[ot][:], o_ps[:], scale)

                # head 0/1 need kt/qt tiles 0..2 first; emit those early
                for ot in (0, 1):
                    project(wkt, mt, kt, None, ot)
                    project(wqt, dtt, qt, SCALE, ot)
                for ot in (2, 3, 4, 5):
                    project(wkt, mt, kt, None, ot)
                    project(wqt, dtt, qt, SCALE, ot)
                # V[s, o] = M @ WV^T : lhsT = Mt (h,s), rhs = WVt (h,o)
                for st in range(ST):
                    v_ps = ps_big.tile([128, H], F32, tag="bigps", name="v_ps")
                    for ht in range(HT):
                        for c0, cw in ((0, 512), (512, 256)):
                            nc.tensor.matmul(
                                v_ps[:, c0:c0 + cw],
                                mt[ht][:, st * 128:(st + 1) * 128],
                                wvt[ht][:, c0:c0 + cw],
                                start=(ht == 0), stop=(ht == HT - 1))
                    nc.scalar.copy(v[st][:], v_ps[:])

            # ---------------- Phase B: attention per head ----------------
            with ExitStack() as pb:
                mkp = pb.enter_context(tc.tile_pool(name="mkp", bufs=1))
                atp = pb.enter_context(tc.tile_pool(name="atp", bufs=2))
                smp = pb.enter_context(tc.tile_pool(name="smp", bufs=3))
                trc = pb.enter_context(tc.tile_pool(name="trc", bufs=3))
                stt = pb.enter_context(tc.tile_pool(name="stt", bufs=8))

                mk_tiles = []
                for q in range(ST):
                    mk = mkp.tile([128, S], U8, tag=f"mk{q}", name=f"mk{q}")
                    nc.sync.dma_start(mk[:], msk[q * 128:(q + 1) * 128, :])
                    mk_tiles.append(mk)
                for i in range(ZT):
                    nc.sync.dma_start(wft[i][:], wf_t[i * 128:(i + 1) * 128, :])
                for st in range(ST):
                    nc.sync.dma_start(d2[st][:], dec[st * 128:(st + 1) * 128, :])

                # broadcast bf/gamma/beta to [128, H] via ones-matmul (early,
                # so the LayerNorm tail has no setup left to wait on)
                bcs = {}
                for nm in ("bf", "ga", "be"):
                    vec = vecs[nm]
                    bc = bcp.tile([128, H], F32, tag=f"bc{nm}", name=f"bc{nm}")
                    for c0, cw in ((0, 512), (512, 256)):
                        bc_ps = ps_av.tile([128, 512], F32, tag="av0a", name="bc_ps")
                        nc.tensor.matmul(bc_ps[:, 0:cw], ones[:],
                                         vec[:, c0:c0 + cw],
                                         start=True, stop=True)
                        nc.scalar.copy(bc[:, c0:c0 + cw], bc_ps[:, 0:cw])
                    bcs[nm] = bc
                for st in range(ST):
                    nc.vector.tensor_add(d2[st][:], d2[st][:], bcs["bf"][:])

                def softmax_tile(h, q, sc_ps, qq_tag):
                    masked = smp.tile([128, S], F32, tag="masked", name="masked")
                    nc.vector.scalar_tensor_tensor(
                        masked[:], mk_tiles[q][:], -BIG, sc_ps[:],
                        OP.mult, OP.add)
                    exp_bf = smp.tile([128, S], BF16, tag="expbf", name="expbf")
                    sums = stt.tile([128, 1], F32, tag="sums", name="sums")
                    nc.scalar.activation(exp_bf[:], masked[:], AF.Exp,
                                         accum_out=sums[:])
                    recip = stt.tile([128, 1], F32, tag="recip", name="recip")
                    nc.vector.reciprocal(recip[:], sums[:])
                    factor = stt.tile([128, 1], F32, tag="factor", name="factor")
                    nc.vector.tensor_mul(factor[:], recip[:], qm_sb[:, q:q + 1])
                    at_bf = atp.tile([128, S], BF16, tag=f"attnbf{qq_tag}",
                                     name=f"attnbf{qq_tag}")
                    nc.vector.tensor_scalar_mul(at_bf[:], exp_bf[:], factor[:])
                    nc.sync.dma_start(out_att[h, q * 128:(q + 1) * 128, :],
                                      at_bf[:])
                    return at_bf

                for h in range(NH):
                    segs = _head_segs(h)
                    for half in range(2):
                        attn_q = []
                        for qq in range(4):
                            q = half * 4 + qq
                            sc_ps = ps_big.tile([128, S], F32, tag="bigps",
                                                name="sc_ps")
                            for si, (t, lo, hi) in enumerate(segs):
                                for c0 in (0, 512):
                                    nc.tensor.matmul(
                                        sc_ps[:, c0:c0 + 512],
                                        qt[t][lo:hi, q * 128:(q + 1) * 128],
                                        kt[t][lo:hi, c0:c0 + 512],
                                        start=(si == 0), stop=(si == len(segs) - 1))
                            attn_q.append(softmax_tile(h, q, sc_ps, str(qq)))

                        av0 = ps_av.tile([128, 512], F32, tag="av0a", name="av0")
                        av1 = ps_av.tile([64, 512], F32, tag="av1p", name="av1")
                        for p in range(ST):
                            tr_ps = ps_tr.tile([128, 512], BF16, tag="trps",
                                               name="trps")
                            for qq in range(4):
                                nc.tensor.transpose(
                                    tr_ps[:, qq * 128:(qq + 1) * 128],
                                    attn_q[qq][:, p * 128:(p + 1) * 128],
                                    ident_bf[:])
                            tr_sb = trc.tile([128, 512], BF16, tag="trsb",
                                             name="trsb")
                            nc.scalar.copy(tr_sb[:], tr_ps[:])
                            nc.tensor.matmul(av0[:], v[p][:, h * DH:h * DH + 128],
                                             tr_sb[:], start=(p == 0),
                                             stop=(p == ST - 1))
                            nc.tensor.matmul(av1[:], v[p][:, h * DH + 128:(h + 1) * DH],
                                             tr_sb[:], start=(p == 0),
                                             stop=(p == ST - 1))
                        _copy_zl_rows(nc, zl, h * DH, av0[:], 128,
                                      half * 512, 512, "v")
                        _copy_zl_rows(nc, zl, h * DH + 128, av1[:], 64,
                                      half * 512, 512, "v")

        # ---------------- Phase C: final linear + LayerNorm --------------
        with ExitStack() as pc:
            fsb = pc.enter_context(tc.tile_pool(name="fsb", bufs=2))
            st2 = pc.enter_context(tc.tile_pool(name="st2", bufs=4))


            zall = dtt + zl
            inv_h = 1.0 / float(H)
            for st in range(ST):
                f_ps = ps_big.tile([128, H], F32, tag="bigps", name="f_ps")
                for zt in range(ZT):
                    for c0, cw in ((0, 512), (512, 256)):
                        nc.tensor.matmul(
                            f_ps[:, c0:c0 + cw],
                            zall[zt][:, st * 128:(st + 1) * 128],
                            wft[zt][:, c0:c0 + cw],
                            start=(zt == 0), stop=(zt == ZT - 1))
                x = fsb.tile([128, H], F32, tag="x", name="x")
                s1 = st2.tile([128, 1], F32, tag="s1", name="s1")
                nc.vector.scalar_tensor_tensor(x[:], f_ps[:], 1.0, d2[st][:],
                                               OP.bypass, OP.add,
                                               accum_out=s1[:])
                mean = st2.tile([128, 1], F32, tag="mean", name="mean")
                nc.vector.tensor_scalar_mul(mean[:], s1[:], inv_h)
                sq = fsb.tile([128, H], F32, tag="sq", name="sq")
                s2 = st2.tile([128, 1], F32, tag="s2", name="s2")
                nc.scalar.activation(sq[:], x[:], AF.Square, accum_out=s2[:])
                ex2 = st2.tile([128, 1], F32, tag="ex2", name="ex2")
                nc.vector.tensor_scalar_mul(ex2[:], s2[:], inv_h)
                m2 = st2.tile([128, 1], F32, tag="m2", name="m2")
                nc.vector.tensor_mul(m2[:], mean[:], mean[:])
                var = st2.tile([128, 1], F32, tag="var", name="var")
                nc.vector.tensor_sub(var[:], ex2[:], m2[:])
                nc.vector.tensor_scalar_add(var[:], var[:], LN_EPS)
                std = st2.tile([128, 1], F32, tag="std", name="std")
                nc.scalar.activation(std[:], var[:], AF.Sqrt)
                rstd = st2.tile([128, 1], F32, tag="rstd", name="rstd")
                nc.vector.reciprocal(rstd[:], std[:])
                nmr = st2.tile([128, 1], F32, tag="nmr", name="nmr")
                nc.vector.tensor_mul(nmr[:], mean[:], rstd[:])
                nc.vector.tensor_scalar_mul(nmr[:], nmr[:], -1.0)
                xn = fsb.tile([128, H], F32, tag="xn", name="xn")
                nc.scalar.activation(xn[:], x[:], AF.Identity,
                                     bias=nmr[:], scale=rstd[:])
                nc.vector.tensor_mul(xn[:], xn[:], bcs["ga"][:])
                nc.vector.tensor_add(xn[:], xn[:], bcs["be"][:])
                nc.sync.dma_start(out_res[st * 128:(st + 1) * 128, :], xn[:])

    nc.compile()
    return nc


_NC = None


def _get_nc():
    global _NC
    if _NC is None:
        _NC = build_kernel()
    return _NC


def _run(inputs, trace=False, **kw):
    import ml_dtypes
    bf16 = ml_dtypes.bfloat16
    B = inputs["memory"].shape[0]
    assert B == N_CORES
    wk_t = np.ascontiguousarray(np.asarray(inputs["WK"]).T).astype(bf16)
    wv_t = np.ascontiguousarray(np.asarray(inputs["WV"]).T).astype(bf16)
    wq_t = np.ascontiguousarray(np.asarray(inputs["WQ"]).T).astype(bf16)
    wf_t = np.ascontiguousarray(np.asarray(inputs["Wf"]).T).astype(bf16)
    in_maps = []
    for b in range(B):
        in_maps.append({
            "memory_T": np.ascontiguousarray(np.asarray(inputs["memory"][b]).T).astype(bf16),
            "decoder_T": np.ascontiguousarray(np.asarray(inputs["decoder_input"][b]).T).astype(bf16),
            "decoder_input": np.ascontiguousarray(inputs["decoder_input"][b]),
            "mask": np.ascontiguousarray(inputs["mask"][b]).view(np.uint8),
            "query_mask": np.ascontiguousarray(inputs["query_mask"][b]),
            "WK_T": wk_t, "WV_T": wv_t, "WQ_T": wq_t, "Wf_T": wf_t,
            "bf": np.asarray(inputs["bf"]).astype(bf16),
            "gamma": np.asarray(inputs["gamma"]).astype(bf16),
            "beta": np.asarray(inputs["beta"]).astype(bf16),
        })
    nc = _get_nc()
    res = run_bass_kernel_spmd(nc, in_maps, core_ids=list(range(N_CORES)),
                               trace=trace, **kw)
    result = np.empty((B, S, H), np.float32)
    attention = np.empty((NH * B // 4, 4, S, S), np.float32)
    att_flat = attention.reshape(NH * B, S, S)
    for b in range(B):
        result[b] = res.results[b]["out_res"]
        for h in range(NH):
            att_flat[h * B + b] = res.results[b]["out_att"][h].astype(np.float32)
    return (result, attention), res


def kernel(**inputs):
    out, _ = _run(inputs, trace=False)
    return out


def kernel_timed(**inputs):
    out, res = _run(inputs, trace=True)
    return out, res


# revision 28
# speedup vs baseline: 1.0054x; 1.0054x over previous
"""Trainium2 Bass kernel for nn_Attention_1992864825947.

Sharding: pure data-parallel over batch (B=8 -> one batch per NeuronCore,
zero collectives).  Each core runs the complete attention block for its
batch; the host shards inputs / gathers outputs.

Layout + precision strategy:
- Host passes pre-transposed, bf16-pre-cast copies of the matmul inputs
  (memory^T, decoder^T, W*^T) - TRN2 fp32 matmul costs 4 cycles/row (two
  half-speed passes) while bf16 costs 1, and PE transposes of raw inputs
  would burn ~70k cycles/core.  Softmax, LayerNorm and both outputs stay
  fp32 in compute (attention DRAM tensor is bf16, widened exactly on host).
- All matmuls contract over the partition axis; softmax/LayerNorm reduce
  over the free axis:
    K^T = WK @ M^T, Q^T = (WQ @ D^T)/sqrt(d), V = M @ WV^T     (PE bf16)
    scores[sq,sk] = Q^T.T @ K^T  per head, PSUM fp32          (PE bf16)
    masked = mask_u8*(-2^32) + scores   (one DVE scalar_tensor_tensor)
    exp + row-sums in one ACT pass (accum_out); no max-subtraction
      needed (scores are bounded ~N(0,1) after the 1/sqrt(d) fold)
    attn_bf16 = exp * (query_mask/sum)  (per-partition scalar, DVE)
    attn^T via PE transposes (bf16, 4 tiles batched per PSUM tile)
    attn @ V with V stationary, accumulated over sk tiles      (PE bf16)
    result = concat(D, attn_out) @ Wf^T + bf + D, then LayerNorm with
      fused residual+rowsum (scalar_tensor_tensor accum_out) and
      rstd/mean folded into one ACT Identity pass.

Measured on trn2 (8 cores, NTFF profile): ~265 us, rel err ~4.6e-3
(bf16-dominated; gate 2e-2).
"""

import numpy as np
from contextlib import ExitStack

import concourse.bass as bass
import concourse.tile as tile
from concourse import bacc, mybir
from concourse.bass_utils import run_bass_kernel_spmd
from concourse.masks import make_identity

F32 = mybir.dt.float32
BF16 = mybir.dt.bfloat16
U8 = mybir.dt.uint8
AF = mybir.ActivationFunctionType
OP = mybir.AluOpType

S = 1024          # sequence length
H = 768           # hidden
NH = 4            # heads
DH = H // NH      # 192 head dim
ST = S // 128     # 8 seq tiles
HT = H // 128     # 6 hidden tiles
ZT = 2 * HT       # 12 tiles of concat dim
BIG = float(2 ** 32)
SCALE = 1.0 / float(np.sqrt(DH))
LN_EPS = 1e-5
N_CORES = 8


def _head_segs(h):
    """Partition-tile segments covering rows [h*DH, (h+1)*DH) of a
    [H, S]-shaped tensor stored as HT tiles of 128 partitions."""
    segs = []
    r = h * DH
    end = (h + 1) * DH
    while r < end:
        t = r // 128
        lo = r % 128
        hi = min(128, lo + (end - r))
        segs.append((t, lo, hi))
        r += hi - lo
    return segs


def _copy_zl_rows(nc, zl, lstart, src, nrows, col_off, width, engine):
    """Copy src[0:nrows, 0:width] (PSUM f32) into logical rows
    [lstart, lstart+nrows) of the Zt-lower tiles (bf16)."""
    r = 0
    while r < nrows:
        t = (lstart + r) // 128
        lo = (lstart + r) % 128
        n = min(128 - lo, nrows - r)
        dst = zl[t][lo:lo + n, col_off:col_off + width]
        if engine == "v":
            nc.vector.tensor_copy(dst, src[r:r + n, 0:width])
        else:
            nc.scalar.copy(dst, src[r:r + n, 0:width])
        r += n


def build_kernel():
    nc = bacc.Bacc("TRN2", target_bir_lowering=False, debug=False,
                   num_devices=N_CORES)
    mem_t = nc.dram_tensor("memory_T", [H, S], BF16, kind="ExternalInput").ap()
    dec_t = nc.dram_tensor("decoder_T", [H, S], BF16, kind="ExternalInput").ap()
    dec = nc.dram_tensor("decoder_input", [S, H], F32, kind="ExternalInput").ap()
    msk = nc.dram_tensor("mask", [S, S], U8, kind="ExternalInput").ap()
    qm = nc.dram_tensor("query_mask", [S], F32, kind="ExternalInput").ap()
    wk_t = nc.dram_tensor("WK_T", [H, H], BF16, kind="ExternalInput").ap()
    wv_t = nc.dram_tensor("WV_T", [H, H], BF16, kind="ExternalInput").ap()
    wq_t = nc.dram_tensor("WQ_T", [H, H], BF16, kind="ExternalInput").ap()
    wf_t = nc.dram_tensor("Wf_T", [2 * H, H], BF16, kind="ExternalInput").ap()
    bfv = nc.dram_tensor("bf", [H], BF16, kind="ExternalInput").ap()
    gav = nc.dram_tensor("gamma", [H], BF16, kind="ExternalInput").ap()
    bev = nc.dram_tensor("beta", [H], BF16, kind="ExternalInput").ap()
    out_res = nc.dram_tensor("out_res", [S, H], F32, kind="ExternalOutput").ap()
    out_att = nc.dram_tensor("out_att", [NH, S, S], BF16, kind="ExternalOutput").ap()

    with tile.TileContext(nc) as tc, ExitStack() as top:
        persist = top.enter_context(tc.tile_pool(name="persist", bufs=1))
        ps_big = top.enter_context(tc.tile_pool(name="ps_big", bufs=2, space="PSUM"))
        ps_av = top.enter_context(tc.tile_pool(name="ps_av", bufs=1, space="PSUM"))
        ps_tr = top.enter_context(tc.tile_pool(name="ps_tr", bufs=2, space="PSUM"))

        ident = persist.tile([128, 128], F32, tag="ident")
        make_identity(nc, ident[:])
        ident_bf = persist.tile([128, 128], BF16, tag="identbf")
        nc.vector.tensor_copy(ident_bf[:], ident[:])
        qm_sb = persist.tile([128, ST], F32, tag="qm")
        nc.sync.dma_start(qm_sb[:], qm.rearrange("(j p) -> p j", p=128))
        dtt = [persist.tile([128, S], BF16, tag=f"dtt{i}", name=f"dtt{i}")
               for i in range(HT)]
        zl = [persist.tile([128, S], BF16, tag=f"zl{i}", name=f"zl{i}")
              for i in range(HT)]
        for i in range(HT):
            for lo in (0, 64):
                nc.sync.dma_start(dtt[i][lo:lo + 64, :],
                                  dec_t[i * 128 + lo:i * 128 + lo + 64, :])

        # Phase C tensors (pools opened before kqv for LIFO release order;
        # loads issued at phase B start as prefetch)
        wfp = top.enter_context(tc.tile_pool(name="wfp", bufs=1))
        d2p = top.enter_context(tc.tile_pool(name="d2p", bufs=1))
        bcp = top.enter_context(tc.tile_pool(name="bcp", bufs=1))
        wft = [wfp.tile([128, H], BF16, tag=f"wft{i}", name=f"wft{i}")
               for i in range(ZT)]
        d2 = [d2p.tile([128, H], F32, tag=f"d2{i}", name=f"d2{i}")
              for i in range(ST)]
        vecs = {}
        for nm, vap in (("bf", bfv), ("ga", gav), ("be", bev)):
            vec = bcp.tile([1, H], BF16, tag=f"vec{nm}", name=f"vec{nm}")
            nc.sync.dma_start(vec[:], vap.rearrange("(p h) -> p h", p=1))
            vecs[nm] = vec
        ones = bcp.tile([1, 128], BF16, tag="ones")
        nc.gpsimd.memset(ones[:], 1.0)

        with ExitStack() as kqv_ctx:
            kqv = kqv_ctx.enter_context(tc.tile_pool(name="kqv", bufs=1))
            kt = [kqv.tile([128, S], BF16, tag=f"kt{i}", name=f"kt{i}")
                  for i in range(HT)]
            qt = [kqv.tile([128, S], BF16, tag=f"qt{i}", name=f"qt{i}")
                  for i in range(HT)]
            v = [kqv.tile([128, H], BF16, tag=f"v{i}", name=f"v{i}")
                 for i in range(ST)]

            # ---------------- Phase A: projections -----------------------
            with ExitStack() as pa:
                mtp = pa.enter_context(tc.tile_pool(name="mtp", bufs=1))
                mt = [mtp.tile([128, S], BF16, tag=f"mt{i}", name=f"mt{i}")
                      for i in range(HT)]
                wkt = [mtp.tile([128, H], BF16, tag=f"wkt{i}", name=f"wkt{i}")
                       for i in range(HT)]
                wqt = [mtp.tile([128, H], BF16, tag=f"wqt{i}", name=f"wqt{i}")
                       for i in range(HT)]
                wvt = [mtp.tile([128, H], BF16, tag=f"wvt{i}", name=f"wvt{i}")
                       for i in range(HT)]
                for i in range(HT):
                    for lo in (0, 64):
                        nc.sync.dma_start(wkt[i][lo:lo + 64, :],
                                          wk_t[i * 128 + lo:i * 128 + lo + 64, :])
                        nc.sync.dma_start(mt[i][lo:lo + 64, :],
                                          mem_t[i * 128 + lo:i * 128 + lo + 64, :])
                for i in range(HT):
                    for lo in (0, 64):
                        nc.sync.dma_start(wqt[i][lo:lo + 64, :],
                                          wq_t[i * 128 + lo:i * 128 + lo + 64, :])
                for i in range(HT):
                    nc.sync.dma_start(wvt[i][:], wv_t[i * 128:(i + 1) * 128, :])

                # K^T/Q^T [o, s] = W @ X^T: lhsT = Wt (h,o), rhs = Xt (h,s)
                def project(wt, src, dst, scale, ot):
                    o_ps = ps_big.tile([128, S], F32, tag="bigps", name="o_ps")
                    for ht in range(HT):
                        for c0 in (0, 512):
                            nc.tensor.matmul(
                                o_ps[:, c0:c0 + 512],
                                wt[ht][:, ot * 128:(ot + 1) * 128],
                                src[ht][:, c0:c0 + 512],
                                start=(ht == 0), stop=(ht == HT - 1))
                    if scale is None:
                        nc.scalar.copy(dst[ot][:], o_ps[:])
                    else:
                        nc.scalar.mul(dst[ot][:], o_ps[:], scale)

                # head 0/1 need kt/qt tiles 0..2 first; emit those early
                for ot in (0, 1):
                    project(wkt, mt, kt, None, ot)
                    project(wqt, dtt, qt, SCALE, ot)
                for ot in (2, 3, 4, 5):
                    project(wkt, mt, kt, None, ot)
                    project(wqt, dtt, qt, SCALE, ot)
                # V[s, o] = M @ WV^T : lhsT = Mt (h,s), rhs = WVt (h,o)
                for st in range(ST):
                    v_ps = ps_big.tile([128, H], F32, tag="bigps", name="v_ps")
                    for ht in range(HT):
                        for c0, cw in ((0, 512), (512, 256)):
                            nc.tensor.matmul(
                                v_ps[:, c0:c0 + cw],
                                mt[ht][:, st * 128:(st + 1) * 128],
                                wvt[ht][:, c0:c0 + cw],
                                start=(ht == 0), stop=(ht == HT - 1))
                    nc.scalar.copy(v[st][:], v_ps[:])

            # ---------------- Phase B: attention per head ----------------
            with ExitStack() as pb:
                mkp = pb.enter_context(tc.tile_pool(name="mkp", bufs=1))
                atp = pb.enter_context(tc.tile_pool(name="atp", bufs=2))
                smp = pb.enter_context(tc.tile_pool(name="smp", bufs=3))
                trc = pb.enter_context(tc.tile_pool(name="trc", bufs=3))
                stt = pb.enter_context(tc.tile_pool(name="stt", bufs=8))

                mk_tiles = []
                for q in range(ST):
                    mk = mkp.tile([128, S], U8, tag=f"mk{q}", name=f"mk{q}")
                    nc.sync.dma_start(mk[:], msk[q * 128:(q + 1) * 128, :])
                    mk_tiles.append(mk)
                for i in range(ZT):
                    nc.sync.dma_start(wft[i][:], wf_t[i * 128:(i + 1) * 128, :])
                for st in range(ST):
                    nc.sync.dma_start(d2[st][:], dec[st * 128:(st + 1) * 128, :])

                # broadcast bf/gamma/beta to [128, H] via ones-matmul (early,
                # so the LayerNorm tail has no setup left to wait on)
                bcs = {}
                for nm in ("bf", "ga", "be"):
                    vec = vecs[nm]
                    bc = bcp.tile([128, H], F32, tag=f"bc{nm}", name=f"bc{nm}")
                    for c0, cw in ((0, 512), (512, 256)):
                        bc_ps = ps_av.tile([128, 512], F32, tag="av0a", name="bc_ps")
                        nc.tensor.matmul(bc_ps[:, 0:cw], ones[:],
                                         vec[:, c0:c0 + cw],
                                         start=True, stop=True)
                        nc.scalar.copy(bc[:, c0:c0 + cw], bc_ps[:, 0:cw])
                    bcs[nm] = bc
                for st in range(ST):
                    nc.vector.tensor_add(d2[st][:], d2[st][:], bcs["bf"][:])

                def softmax_tile(h, q, sc_ps, qq_tag):
                    masked = smp.tile([128, S], F32, tag="masked", name="masked")
                    nc.vector.scalar_tensor_tensor(
                        masked[:], mk_tiles[q][:], -BIG, sc_ps[:],
                        OP.mult, OP.add)
                    exp_bf = smp.tile([128, S], BF16, tag="expbf", name="expbf")
                    sums = stt.tile([128, 1], F32, tag="sums", name="sums")
                    nc.scalar.activation(exp_bf[:], masked[:], AF.Exp,
                                         accum_out=sums[:])
                    recip = stt.tile([128, 1], F32, tag="recip", name="recip")
                    nc.vector.reciprocal(recip[:], sums[:])
                    factor = stt.tile([128, 1], F32, tag="factor", name="factor")
                    nc.vector.tensor_mul(factor[:], recip[:], qm_sb[:, q:q + 1])
                    at_bf = atp.tile([128, S], BF16, tag=f"attnbf{qq_tag}",
                                     name=f"attnbf{qq_tag}")
                    nc.vector.tensor_scalar_mul(at_bf[:], exp_bf[:], factor[:])
                    nc.sync.dma_start(out_att[h, q * 128:(q + 1) * 128, :],
                                      at_bf[:])
                    return at_bf

                for h in range(NH):
                    segs = _head_segs(h)
                    for half in range(2):
                        attn_q = []
                        for qq in range(4):
                            q = half * 4 + qq
                            sc_ps = ps_big.tile([128, S], F32, tag="bigps",
                                                name="sc_ps")
                            for si, (t, lo, hi) in enumerate(segs):
                                for c0 in (0, 512):
                                    nc.tensor.matmul(
                                        sc_ps[:, c0:c0 + 512],
                                        qt[t][lo:hi, q * 128:(q + 1) * 128],
                                        kt[t][lo:hi, c0:c0 + 512],
                                        start=(si == 0), stop=(si == len(segs) - 1))
                            attn_q.append(softmax_tile(h, q, sc_ps, str(qq)))

                        av0 = ps_av.tile([128, 512], F32, tag="av0a", name="av0")
                        av1 = ps_av.tile([64, 512], F32, tag="av1p", name="av1")
                        for p in range(ST):
                            tr_ps = ps_tr.tile([128, 512], BF16, tag="trps",
                                               name="trps")
                            for qq in range(4):
                                nc.tensor.transpose(
                                    tr_ps[:, qq * 128:(qq + 1) * 128],
                                    attn_q[qq][:, p * 128:(p + 1) * 128],
                                    ident_bf[:])
                            tr_sb = trc.tile([128, 512], BF16, tag="trsb",
                                             name="trsb")
                            nc.vector.tensor_copy(tr_sb[:], tr_ps[:])
                            nc.tensor.matmul(av0[:], v[p][:, h * DH:h * DH + 128],
                                             tr_sb[:], start=(p == 0),
                                             stop=(p == ST - 1))
                            nc.tensor.matmul(av1[:], v[p][:, h * DH + 128:(h + 1) * DH],
                                             tr_sb[:], start=(p == 0),
                                             stop=(p == ST - 1))
                        _copy_zl_rows(nc, zl, h * DH, av0[:], 128,
                                      half * 512, 512, "v")
                        _copy_zl_rows(nc, zl, h * DH + 128, av1[:], 64,
                                      half * 512, 512, "v")

        # ---------------- Phase C: final linear + LayerNorm --------------
        with ExitStack() as pc:
            fsb = pc.enter_context(tc.tile_pool(name="fsb", bufs=2))
            st2 = pc.enter_context(tc.tile_pool(name="st2", bufs=4))


            zall = dtt + zl
            inv_h = 1.0 / float(H)
            for st in range(ST):
                f_ps = ps_big.tile([128, H], F32, tag="bigps", name="f_ps")
                for zt in range(ZT):
                    for c0, cw in ((0, 512), (512, 256)):
                        nc.tensor.matmul(
                            f_ps[:, c0:c0 + cw],
                            zall[zt][:, st * 128:(st + 1) * 128],
                            wft[zt][:, c0:c0 + cw],
                            start=(zt == 0), stop=(zt == ZT - 1))
                x = fsb.tile([128, H], F32, tag="x", name="x")
                s1 = st2.tile([128, 1], F32, tag="s1", name="s1")
                nc.vector.scalar_tensor_tensor(x[:], f_ps[:], 1.0, d2[st][:],
                                               OP.bypass, OP.add,
                                               accum_out=s1[:])
                mean = st2.tile([128, 1], F32, tag="mean", name="mean")
                nc.vector.tensor_scalar_mul(mean[:], s1[:], inv_h)
                sq = fsb.tile([128, H], F32, tag="sq", name="sq")
                s2 = st2.tile([128, 1], F32, tag="s2", name="s2")
                nc.scalar.activation(sq[:], x[:], AF.Square, accum_out=s2[:])
                ex2 = st2.tile([128, 1], F32, tag="ex2", name="ex2")
                nc.vector.tensor_scalar_mul(ex2[:], s2[:], inv_h)
                m2 = st2.tile([128, 1], F32, tag="m2", name="m2")
                nc.vector.tensor_mul(m2[:], mean[:], mean[:])
                var = st2.tile([128, 1], F32, tag="var", name="var")
                nc.vector.tensor_sub(var[:], ex2[:], m2[:])
                nc.vector.tensor_scalar_add(var[:], var[:], LN_EPS)
                std = st2.tile([128, 1], F32, tag="std", name="std")
                nc.scalar.activation(std[:], var[:], AF.Sqrt)
                rstd = st2.tile([128, 1], F32, tag="rstd", name="rstd")
                nc.vector.reciprocal(rstd[:], std[:])
                nmr = st2.tile([128, 1], F32, tag="nmr", name="nmr")
                nc.vector.tensor_mul(nmr[:], mean[:], rstd[:])
                nc.vector.tensor_scalar_mul(nmr[:], nmr[:], -1.0)
                xn = fsb.tile([128, H], F32, tag="xn", name="xn")
                nc.scalar.activation(xn[:], x[:], AF.Identity,
                                     bias=nmr[:], scale=rstd[:])
                nc.vector.tensor_mul(xn[:], xn[:], bcs["ga"][:])
                nc.vector.tensor_add(xn[:], xn[:], bcs["be"][:])
                nc.sync.dma_start(out_res[st * 128:(st + 1) * 128, :], xn[:])

    nc.compile()
    return nc


_NC = None


def _get_nc():
    global _NC
    if _NC is None:
        _NC = build_kernel()
    return _NC


def _run(inputs, trace=False, **kw):
    import ml_dtypes
    bf16 = ml_dtypes.bfloat16
    B = inputs["memory"].shape[0]
    assert B == N_CORES
    wk_t = np.ascontiguousarray(np.asarray(inputs["WK"]).T).astype(bf16)
    wv_t = np.ascontiguousarray(np.asarray(inputs["WV"]).T).astype(bf16)
    wq_t = np.ascontiguousarray(np.asarray(inputs["WQ"]).T).astype(bf16)
    wf_t = np.ascontiguousarray(np.asarray(inputs["Wf"]).T).astype(bf16)
    in_maps = []
    for b in range(B):
        in_maps.append({
            "memory_T": np.ascontiguousarray(np.asarray(inputs["memory"][b]).T).astype(bf16),
            "decoder_T": np.ascontiguousarray(np.asarray(inputs["decoder_input"][b]).T).astype(bf16),
            "decoder_input": np.ascontiguousarray(inputs["decoder_input"][b]),
            "mask": np.ascontiguousarray(inputs["mask"][b]).view(np.uint8),
            "query_mask": np.ascontiguousarray(inputs["query_mask"][b]),
            "WK_T": wk_t, "WV_T": wv_t, "WQ_T": wq_t, "Wf_T": wf_t,
            "bf": np.asarray(inputs["bf"]).astype(bf16),
            "gamma": np.asarray(inputs["gamma"]).astype(bf16),
            "beta": np.asarray(inputs["beta"]).astype(bf16),
        })
    nc = _get_nc()
    res = run_bass_kernel_spmd(nc, in_maps, core_ids=list(range(N_CORES)),
                               trace=trace, **kw)
    result = np.empty((B, S, H), np.float32)
    attention = np.empty((NH * B // 4, 4, S, S), np.float32)
    att_flat = attention.reshape(NH * B, S, S)
    for b in range(B):
        result[b] = res.results[b]["out_res"]
        for h in range(NH):
            att_flat[h * B + b] = res.results[b]["out_att"][h].astype(np.float32)
    return (result, attention), res


def kernel(**inputs):
    out, _ = _run(inputs, trace=False)
    return out


def kernel_timed(**inputs):
    out, res = _run(inputs, trace=True)
    return out, res


# revision 30
# speedup vs baseline: 1.0630x; 1.0573x over previous
"""Trainium2 Bass kernel for nn_Attention_1992864825947.

Sharding: pure data-parallel over batch (B=8 -> one batch per NeuronCore,
zero collectives).  Each core runs the complete attention block for its
batch; the host shards inputs / gathers outputs.

Layout + precision strategy:
- Host passes pre-transposed, bf16-pre-cast copies of the matmul inputs
  (memory^T, decoder^T, W*^T) - TRN2 fp32 matmul costs 4 cycles/row (two
  half-speed passes) while bf16 costs 1, and PE transposes of raw inputs
  would burn ~70k cycles/core.  Softmax, LayerNorm and both outputs stay
  fp32 in compute (attention DRAM tensor is bf16, widened exactly on host).
- All matmuls contract over the partition axis; softmax/LayerNorm reduce
  over the free axis:
    K^T = WK @ M^T, Q^T = (WQ @ D^T)/sqrt(d), V = M @ WV^T     (PE bf16)
    scores[sq,sk] = Q^T.T @ K^T  per head, PSUM fp32          (PE bf16)
    masked = mask_u8*(-2^32) + scores   (one DVE scalar_tensor_tensor)
    exp + row-sums in one ACT pass (accum_out); no max-subtraction
      needed (scores are bounded ~N(0,1) after the 1/sqrt(d) fold)
    attn_bf16 = exp * (query_mask/sum)  (per-partition scalar, DVE)
    attn^T via PE transposes (bf16, 4 tiles batched per PSUM tile)
    attn @ V with V stationary, accumulated over sk tiles      (PE bf16)
    result = concat(D, attn_out) @ Wf^T + bf + D, then LayerNorm with
      fused residual+rowsum (scalar_tensor_tensor accum_out) and
      rstd/mean folded into one ACT Identity pass.

Measured on trn2 (8 cores, NTFF profile): ~265 us, rel err ~4.6e-3
(bf16-dominated; gate 2e-2).
"""

import numpy as np
from contextlib import ExitStack

import concourse.bass as bass
import concourse.tile as tile
from concourse import bacc, mybir
from concourse.bass_utils import run_bass_kernel_spmd
from concourse.masks import make_identity

F32 = mybir.dt.float32
BF16 = mybir.dt.bfloat16
U8 = mybir.dt.uint8
AF = mybir.ActivationFunctionType
OP = mybir.AluOpType

S = 1024          # sequence length
H = 768           # hidden
NH = 4            # heads
DH = H // NH      # 192 head dim
ST = S // 128     # 8 seq tiles
HT = H // 128     # 6 hidden tiles
ZT = 2 * HT       # 12 tiles of concat dim
BIG = float(2 ** 32)
SCALE = 1.0 / float(np.sqrt(DH))
LN_EPS = 1e-5
N_CORES = 8


def _head_segs(h):
    """Partition-tile segments covering rows [h*DH, (h+1)*DH) of a
    [H, S]-shaped tensor stored as HT tiles of 128 partitions."""
    segs = []
    r = h * DH
    end = (h + 1) * DH
    while r < end:
        t = r // 128
        lo = r % 128
        hi = min(128, lo + (end - r))
        segs.append((t, lo, hi))
        r += hi - lo
    return segs


def _copy_zl_rows(nc, zl, lstart, src, nrows, col_off, width, engine):
    """Copy src[0:nrows, 0:width] (PSUM f32) into logical rows
    [lstart, lstart+nrows) of the Zt-lower tiles (bf16)."""
    r = 0
    while r < nrows:
        t = (lstart + r) // 128
        lo = (lstart + r) % 128
        n = min(128 - lo, nrows - r)
        dst = zl[t][lo:lo + n, col_off:col_off + width]
        if engine == "v":
            nc.vector.tensor_copy(dst, src[r:r + n, 0:width])
        else:
            nc.scalar.copy(dst, src[r:r + n, 0:width])
        r += n


def build_kernel():
    nc = bacc.Bacc("TRN2", target_bir_lowering=False, debug=False,
                   num_devices=N_CORES)
    mem_t = nc.dram_tensor("memory_T", [H, S], BF16, kind="ExternalInput").ap()
    dec_t = nc.dram_tensor("decoder_T", [H, S], BF16, kind="ExternalInput").ap()
    dec = nc.dram_tensor("decoder_input", [S, H], F32, kind="ExternalInput").ap()
    msk = nc.dram_tensor("mask", [S, S], U8, kind="ExternalInput").ap()
    qm = nc.dram_tensor("query_mask", [S], F32, kind="ExternalInput").ap()
    wk_t = nc.dram_tensor("WK_T", [H, H], BF16, kind="ExternalInput").ap()
    wv_t = nc.dram_tensor("WV_T", [H, H], BF16, kind="ExternalInput").ap()
    wq_t = nc.dram_tensor("WQ_T", [H, H], BF16, kind="ExternalInput").ap()
    wf_t = nc.dram_tensor("Wf_T", [2 * H, H], BF16, kind="ExternalInput").ap()
    bfv = nc.dram_tensor("bf", [H], BF16, kind="ExternalInput").ap()
    gav = nc.dram_tensor("gamma", [H], BF16, kind="ExternalInput").ap()
    bev = nc.dram_tensor("beta", [H], BF16, kind="ExternalInput").ap()
    out_res = nc.dram_tensor("out_res", [S, H], F32, kind="ExternalOutput").ap()
    out_att = nc.dram_tensor("out_att", [NH, S, S], BF16, kind="ExternalOutput").ap()

    with tile.TileContext(nc) as tc, ExitStack() as top:
        persist = top.enter_context(tc.tile_pool(name="persist", bufs=1))
        ps_big = top.enter_context(tc.tile_pool(name="ps_big", bufs=2, space="PSUM"))
        ps_av = top.enter_context(tc.tile_pool(name="ps_av", bufs=1, space="PSUM"))
        ps_tr = top.enter_context(tc.tile_pool(name="ps_tr", bufs=2, space="PSUM"))

        ident = persist.tile([128, 128], F32, tag="ident")
        make_identity(nc, ident[:])
        ident_bf = persist.tile([128, 128], BF16, tag="identbf")
        nc.vector.tensor_copy(ident_bf[:], ident[:])
        qm_sb = persist.tile([128, ST], F32, tag="qm")
        nc.sync.dma_start(qm_sb[:], qm.rearrange("(j p) -> p j", p=128))
        dtt = [persist.tile([128, S], BF16, tag=f"dtt{i}", name=f"dtt{i}")
               for i in range(HT)]
        zl = [persist.tile([128, S], BF16, tag=f"zl{i}", name=f"zl{i}")
              for i in range(HT)]
        for i in range(HT):
            nc.sync.dma_start(dtt[i][:], dec_t[i * 128:(i + 1) * 128, :])

        # Phase C tensors (pools opened before kqv for LIFO release order;
        # loads issued at phase B start as prefetch)
        wfp = top.enter_context(tc.tile_pool(name="wfp", bufs=1))
        d2p = top.enter_context(tc.tile_pool(name="d2p", bufs=1))
        bcp = top.enter_context(tc.tile_pool(name="bcp", bufs=1))
        wft = [wfp.tile([128, H], BF16, tag=f"wft{i}", name=f"wft{i}")
               for i in range(ZT)]
        d2 = [d2p.tile([128, H], F32, tag=f"d2{i}", name=f"d2{i}")
              for i in range(ST)]
        vecs = {}
        for nm, vap in (("bf", bfv), ("ga", gav), ("be", bev)):
            vec = bcp.tile([1, H], BF16, tag=f"vec{nm}", name=f"vec{nm}")
            nc.sync.dma_start(vec[:], vap.rearrange("(p h) -> p h", p=1))
            vecs[nm] = vec
        ones = bcp.tile([1, 128], BF16, tag="ones")
        nc.gpsimd.memset(ones[:], 1.0)

        with ExitStack() as kqv_ctx:
            kqv = kqv_ctx.enter_context(tc.tile_pool(name="kqv", bufs=1))
            kt = [kqv.tile([128, S], BF16, tag=f"kt{i}", name=f"kt{i}")
                  for i in range(HT)]
            qt = [kqv.tile([128, S], BF16, tag=f"qt{i}", name=f"qt{i}")
                  for i in range(HT)]
            v = [kqv.tile([128, H], BF16, tag=f"v{i}", name=f"v{i}")
                 for i in range(ST)]

            # ---------------- Phase A: projections -----------------------
            with ExitStack() as pa:
                mtp = pa.enter_context(tc.tile_pool(name="mtp", bufs=1))
                mt = [mtp.tile([128, S], BF16, tag=f"mt{i}", name=f"mt{i}")
                      for i in range(HT)]
                wkt = [mtp.tile([128, H], BF16, tag=f"wkt{i}", name=f"wkt{i}")
                       for i in range(HT)]
                wqt = [mtp.tile([128, H], BF16, tag=f"wqt{i}", name=f"wqt{i}")
                       for i in range(HT)]
                wvt = [mtp.tile([128, H], BF16, tag=f"wvt{i}", name=f"wvt{i}")
                       for i in range(HT)]
                for i in range(HT):
                    nc.sync.dma_start(wkt[i][:], wk_t[i * 128:(i + 1) * 128, :])
                    nc.sync.dma_start(mt[i][:], mem_t[i * 128:(i + 1) * 128, :])
                for i in range(HT):
                    nc.sync.dma_start(wqt[i][:], wq_t[i * 128:(i + 1) * 128, :])
                for i in range(HT):
                    nc.sync.dma_start(wvt[i][:], wv_t[i * 128:(i + 1) * 128, :])

                # K^T/Q^T [o, s] = W @ X^T: lhsT = Wt (h,o), rhs = Xt (h,s)
                def project(wt, src, dst, scale, ot):
                    o_ps = ps_big.tile([128, S], F32, tag="bigps", name="o_ps")
                    for ht in range(HT):
                        for c0 in (0, 512):
                            nc.tensor.matmul(
                                o_ps[:, c0:c0 + 512],
                                wt[ht][:, ot * 128:(ot + 1) * 128],
                                src[ht][:, c0:c0 + 512],
                                start=(ht == 0), stop=(ht == HT - 1))
                    if scale is None:
                        nc.scalar.copy(dst[ot][:], o_ps[:])
                    else:
                        nc.scalar.mul(dst[ot][:], o_ps[:], scale)

                # head 0/1 need kt/qt tiles 0..2 first; emit those early
                for ot in (0, 1):
                    project(wkt, mt, kt, None, ot)
                    project(wqt, dtt, qt, SCALE, ot)
                for ot in (2, 3, 4, 5):
                    project(wkt, mt, kt, None, ot)
                    project(wqt, dtt, qt, SCALE, ot)
                # V[s, o] = M @ WV^T : lhsT = Mt (h,s), rhs = WVt (h,o)
                for st in range(ST):
                    v_ps = ps_big.tile([128, H], F32, tag="bigps", name="v_ps")
                    for ht in range(HT):
                        for c0, cw in ((0, 512), (512, 256)):
                            nc.tensor.matmul(
                                v_ps[:, c0:c0 + cw],
                                mt[ht][:, st * 128:(st + 1) * 128],
                                wvt[ht][:, c0:c0 + cw],
                                start=(ht == 0), stop=(ht == HT - 1))
                    nc.scalar.copy(v[st][:], v_ps[:])

            # ---------------- Phase B: attention per head ----------------
            with ExitStack() as pb:
                mkp = pb.enter_context(tc.tile_pool(name="mkp", bufs=1))
                atp = pb.enter_context(tc.tile_pool(name="atp", bufs=2))
                smp = pb.enter_context(tc.tile_pool(name="smp", bufs=3))
                trc = pb.enter_context(tc.tile_pool(name="trc", bufs=3))
                stt = pb.enter_context(tc.tile_pool(name="stt", bufs=8))

                mk_tiles = []
                for q in range(ST):
                    mk = mkp.tile([128, S], U8, tag=f"mk{q}", name=f"mk{q}")
                    nc.sync.dma_start(mk[:], msk[q * 128:(q + 1) * 128, :])
                    mk_tiles.append(mk)
                for i in range(ZT):
                    nc.sync.dma_start(wft[i][:], wf_t[i * 128:(i + 1) * 128, :])
                for st in range(ST):
                    nc.sync.dma_start(d2[st][:], dec[st * 128:(st + 1) * 128, :])

                # broadcast bf/gamma/beta to [128, H] via ones-matmul (early,
                # so the LayerNorm tail has no setup left to wait on)
                bcs = {}
                for nm in ("bf", "ga", "be"):
                    vec = vecs[nm]
                    bc = bcp.tile([128, H], F32, tag=f"bc{nm}", name=f"bc{nm}")
                    for c0, cw in ((0, 512), (512, 256)):
                        bc_ps = ps_av.tile([128, 512], F32, tag="av0a", name="bc_ps")
                        nc.tensor.matmul(bc_ps[:, 0:cw], ones[:],
                                         vec[:, c0:c0 + cw],
                                         start=True, stop=True)
                        nc.scalar.copy(bc[:, c0:c0 + cw], bc_ps[:, 0:cw])
                    bcs[nm] = bc
                for st in range(ST):
                    nc.vector.tensor_add(d2[st][:], d2[st][:], bcs["bf"][:])

                def softmax_tile(h, q, sc_ps, qq_tag):
                    masked = smp.tile([128, S], F32, tag="masked", name="masked")
                    nc.vector.scalar_tensor_tensor(
                        masked[:], mk_tiles[q][:], -BIG, sc_ps[:],
                        OP.mult, OP.add)
                    exp_bf = smp.tile([128, S], BF16, tag="expbf", name="expbf")
                    sums = stt.tile([128, 1], F32, tag="sums", name="sums")
                    nc.scalar.activation(exp_bf[:], masked[:], AF.Exp,
                                         accum_out=sums[:])
                    recip = stt.tile([128, 1], F32, tag="recip", name="recip")
                    nc.vector.reciprocal(recip[:], sums[:])
                    factor = stt.tile([128, 1], F32, tag="factor", name="factor")
                    nc.vector.tensor_mul(factor[:], recip[:], qm_sb[:, q:q + 1])
                    at_bf = atp.tile([128, S], BF16, tag=f"attnbf{qq_tag}",
                                     name=f"attnbf{qq_tag}")
                    nc.vector.tensor_scalar_mul(at_bf[:], exp_bf[:], factor[:])
                    nc.sync.dma_start(out_att[h, q * 128:(q + 1) * 128, :],
                                      at_bf[:])
                    return at_bf

                for h in range(NH):
                    segs = _head_segs(h)
                    for half in range(2):
                        attn_q = []
                        for qq in range(4):
                            q = half * 4 + qq
                            sc_ps = ps_big.tile([128, S], F32, tag="bigps",
                                                name="sc_ps")
                            for si, (t, lo, hi) in enumerate(segs):
                                for c0 in (0, 512):
                                    nc.tensor.matmul(
                                        sc_ps[:, c0:c0 + 512],
                                        qt[t][lo:hi, q * 128:(q + 1) * 128],
                                        kt[t][lo:hi, c0:c0 + 512],
                                        start=(si == 0), stop=(si == len(segs) - 1))
                            attn_q.append(softmax_tile(h, q, sc_ps, str(qq)))

                        av0 = ps_av.tile([128, 512], F32, tag="av0a", name="av0")
                        av1 = ps_av.tile([64, 512], F32, tag="av1p", name="av1")
                        for p in range(ST):
                            tr_ps = ps_tr.tile([128, 512], BF16, tag="trps",
                                               name="trps")
                            for qq in range(4):
                                nc.tensor.transpose(
                                    tr_ps[:, qq * 128:(qq + 1) * 128],
                                    attn_q[qq][:, p * 128:(p + 1) * 128],
                                    ident_bf[:])
                            tr_sb = trc.tile([128, 512], BF16, tag="trsb",
                                             name="trsb")
                            nc.vector.tensor_copy(tr_sb[:], tr_ps[:])
                            nc.tensor.matmul(av0[:], v[p][:, h * DH:h * DH + 128],
                                             tr_sb[:], start=(p == 0),
                                             stop=(p == ST - 1))
                            nc.tensor.matmul(av1[:], v[p][:, h * DH + 128:(h + 1) * DH],
                                             tr_sb[:], start=(p == 0),
                                             stop=(p == ST - 1))
                        _copy_zl_rows(nc, zl, h * DH, av0[:], 128,
                                      half * 512, 512, "v")
                        _copy_zl_rows(nc, zl, h * DH + 128, av1[:], 64,
                                      half * 512, 512, "v")

        # ---------------- Phase C: final linear + LayerNorm --------------
        with ExitStack() as pc:
            fsb = pc.enter_context(tc.tile_pool(name="fsb", bufs=2))
            st2 = pc.enter_context(tc.tile_pool(name="st2", bufs=4))


            zall = dtt + zl
            inv_h = 1.0 / float(H)
            for st in range(ST):
                f_ps = ps_big.tile([128, H], F32, tag="bigps", name="f_ps")
                for zt in range(ZT):
                    for c0, cw in ((0, 512), (512, 256)):
                        nc.tensor.matmul(
                            f_ps[:, c0:c0 + cw],
                            zall[zt][:, st * 128:(st + 1) * 128],
                            wft[zt][:, c0:c0 + cw],
                            start=(zt == 0), stop=(zt == ZT - 1))
                x = fsb.tile([128, H], F32, tag="x", name="x")
                s1 = st2.tile([128, 1], F32, tag="s1", name="s1")
                nc.vector.scalar_tensor_tensor(x[:], f_ps[:], 1.0, d2[st][:],
                                               OP.bypass, OP.add,
                                               accum_out=s1[:])
                mean = st2.tile([128, 1], F32, tag="mean", name="mean")
                nc.vector.tensor_scalar_mul(mean[:], s1[:], inv_h)
                sq = fsb.tile([128, H], F32, tag="sq", name="sq")
                s2 = st2.tile([128, 1], F32, tag="s2", name="s2")
                nc.scalar.activation(sq[:], x[:], AF.Square, accum_out=s2[:])
                ex2 = st2.tile([128, 1], F32, tag="ex2", name="ex2")
                nc.vector.tensor_scalar_mul(ex2[:], s2[:], inv_h)
                m2 = st2.tile([128, 1], F32, tag="m2", name="m2")
                nc.vector.tensor_mul(m2[:], mean[:], mean[:])
                var = st2.tile([128, 1], F32, tag="var", name="var")
                nc.vector.tensor_sub(var[:], ex2[:], m2[:])
                nc.vector.tensor_scalar_add(var[:], var[:], LN_EPS)
                std = st2.tile([128, 1], F32, tag="std", name="std")
                nc.scalar.activation(std[:], var[:], AF.Sqrt)
                rstd = st2.tile([128, 1], F32, tag="rstd", name="rstd")
                nc.vector.reciprocal(rstd[:], std[:])
                nmr = st2.tile([128, 1], F32, tag="nmr", name="nmr")
                nc.vector.tensor_mul(nmr[:], mean[:], rstd[:])
                nc.vector.tensor_scalar_mul(nmr[:], nmr[:], -1.0)
                xn = fsb.tile([128, H], F32, tag="xn", name="xn")
                nc.scalar.activation(xn[:], x[:], AF.Identity,
                                     bias=nmr[:], scale=rstd[:])
                nc.vector.tensor_mul(xn[:], xn[:], bcs["ga"][:])
                nc.gpsimd.tensor_add(xn[:], xn[:], bcs["be"][:])
                nc.sync.dma_start(out_res[st * 128:(st + 1) * 128, :], xn[:])

    nc.compile()
    return nc


_NC = None


def _get_nc():
    global _NC
    if _NC is None:
        _NC = build_kernel()
    return _NC


def _run(inputs, trace=False, **kw):
    import ml_dtypes
    bf16 = ml_dtypes.bfloat16
    B = inputs["memory"].shape[0]
    assert B == N_CORES
    wk_t = np.ascontiguousarray(np.asarray(inputs["WK"]).T).astype(bf16)
    wv_t = np.ascontiguousarray(np.asarray(inputs["WV"]).T).astype(bf16)
    wq_t = np.ascontiguousarray(np.asarray(inputs["WQ"]).T).astype(bf16)
    wf_t = np.ascontiguousarray(np.asarray(inputs["Wf"]).T).astype(bf16)
    in_maps = []
    for b in range(B):
        in_maps.append({
            "memory_T": np.ascontiguousarray(np.asarray(inputs["memory"][b]).T).astype(bf16),
            "decoder_T": np.ascontiguousarray(np.asarray(inputs["decoder_input"][b]).T).astype(bf16),
            "decoder_input": np.ascontiguousarray(inputs["decoder_input"][b]),
            "mask": np.ascontiguousarray(inputs["mask"][b]).view(np.uint8),
            "query_mask": np.ascontiguousarray(inputs["query_mask"][b]),
            "WK_T": wk_t, "WV_T": wv_t, "WQ_T": wq_t, "Wf_T": wf_t,
            "bf": np.asarray(inputs["bf"]).astype(bf16),
            "gamma": np.asarray(inputs["gamma"]).astype(bf16),
            "beta": np.asarray(inputs["beta"]).astype(bf16),
        })
    nc = _get_nc()
    res = run_bass_kernel_spmd(nc, in_maps, core_ids=list(range(N_CORES)),
                               trace=trace, **kw)
    result = np.empty((B, S, H), np.float32)
    attention = np.empty((NH * B // 4, 4, S, S), np.float32)
    att_flat = attention.reshape(NH * B, S, S)
    for b in range(B):
        result[b] = res.results[b]["out_res"]
        for h in range(NH):
            att_flat[h * B + b] = res.results[b]["out_att"][h].astype(np.float32)
    return (result, attention), res


def kernel(**inputs):
    out, _ = _run(inputs, trace=False)
    return out


def kernel_timed(**inputs):
    out, res = _run(inputs, trace=True)
    return out, res
